# revision 26
# baseline (speedup 1.0000x reference)
"""Trainium2 Bass kernel for nn_AtlasMultiDiffAttn (8-core data-parallel).

Self-contained: hardcodes shapes (x [8192,56,128] f32 -> out [8192,56] f32).
Per core: 1024 samples, 8 tiles of BT=128 samples (64 even/odd sample pairs).

v2: fp8 (e4m3, TRN +-240) DoubleRow convs.
  - conv1/conv2 matmuls run fp8 DoubleRow (2 taps per instruction, Ko step=1
    overlapping-window rhs - verified on HW), 0.5 cyc/row.
  - x arrives in two host layouts: pair-image xe [pair, row, 160] fp8 for
    conv1 (rows 0-55 even / 64-119 odd sample channels, col c = x[., c-3]),
    and xt [pair, e, 2, 56] fp8 for k-proj (no on-device x transposes).
  - scales: w1 x16, w2 x16, wk x8 keep fp8 weights out of subnormals;
    h = 32*silu (<240), k_T = 8*k; layernorms absorb scales via
    EPS_Q = (1024*56)^2*EPS and EPS_K = 64*EPS.
  - silu via tanh (exp_and_others act table; no table switches):
    2silu(y) = (tanh(y/2)+1)*y with y held scaled in PSUM.
  - h_T via PE transposes (fp8); stats/scores as in v1 but fp8/bf16 operands.
"""
from contextlib import ExitStack

import numpy as np

import concourse.bass as bass
import concourse.tile as tile
from concourse import bacc, mybir
from concourse.bass_utils import run_bass_kernel_spmd

F32 = mybir.dt.float32
BF16 = mybir.dt.bfloat16
F8 = mybir.dt.float8e4
I32 = mybir.dt.int32
AF = mybir.ActivationFunctionType
OP = mybir.AluOpType
AX = mybir.AxisListType
DR = mybir.MatmulPerfMode.DoubleRow

B, A, E = 8192, 56, 128
H, HD = 4, 16
LAMBDA_INIT = 0.7
EPS = 1e-5
SCALING = HD ** -0.5

W1S = 16.0                # conv1 weight quant scale
W2S = 16.0                # conv2 weight quant scale
WKS = 8.0                 # k-proj weight quant scale
QC_SCALE = 4.0 * W1S * W2S * A   # q_acc = QC_SCALE * q_true
EPS_Q = QC_SCALE * QC_SCALE * EPS
EPS_K = WKS * WKS * EPS

NCORES = 8
NB = B // NCORES          # 1024 samples per core
BT = 128                  # samples per tile
NT = NB // BT             # 8 tiles
NPAIR = BT // 2           # 64
XW = 136                  # padded conv1 col count (cols 3..130 = x)
MAGIC = 0x5F3759DF

WSPEC = {}
HAS_QC = True


def _newton_rsqrt(nc, pool, v_ap, shape, tag):
    """v_ap <- rsqrt(v_ap): magic init + 1 Newton iteration (~0.2% rel)."""
    y = pool.tile(list(shape), F32, tag=f"nwy_{tag}")
    t = pool.tile(list(shape), F32, tag=f"nwt_{tag}")
    npart = v_ap.shape[0]
    ya, ta = y[0:npart], t[0:npart]
    nc.vector.tensor_scalar(out=ya.bitcast(I32), in0=v_ap.bitcast(I32),
                            scalar1=1, scalar2=None,
                            op0=OP.logical_shift_right)
    nc.vector.tensor_scalar(out=ya.bitcast(I32), in0=ya.bitcast(I32),
                            scalar1=-1, scalar2=MAGIC,
                            op0=OP.mult, op1=OP.add)
    nc.vector.tensor_tensor(out=ta, in0=ya, in1=ya, op=OP.mult)
    nc.vector.tensor_tensor(out=ta, in0=ta, in1=v_ap, op=OP.mult)
    nc.vector.tensor_scalar(out=ta, in0=ta, scalar1=-0.5, scalar2=1.5,
                            op0=OP.mult, op1=OP.add)
    nc.vector.tensor_tensor(out=v_ap, in0=ya, in1=ta, op=OP.mult)


def apx(base, insert, offset_add):
    """Append (stride,count) dims to an AP and bump its offset."""
    return bass.AP(tensor=base.tensor, offset=base.offset + offset_add,
                   ap=list(base.ap) + [list(d) for d in insert])


def build_tile_kernel(ctx, tc, xe_ext, xt_ext, out_ext, wext, reps=1):
    nc = tc.nc

    consts = ctx.enter_context(tc.tile_pool(name="consts", bufs=1))
    sbX = ctx.enter_context(tc.tile_pool(name="sbX", bufs=2))
    sbT = ctx.enter_context(tc.tile_pool(name="sbT", bufs=2))
    sbH = ctx.enter_context(tc.tile_pool(name="sbH", bufs=1))
    sbK = ctx.enter_context(tc.tile_pool(name="sbK", bufs=2))
    sbS = ctx.enter_context(tc.tile_pool(name="sbS", bufs=2))
    sb2 = ctx.enter_context(tc.tile_pool(name="sb2", bufs=2))
    sbQ = ctx.enter_context(tc.tile_pool(name="sbQ", bufs=1))
    psA = ctx.enter_context(tc.tile_pool(name="psA", bufs=2, space="PSUM"))
    psB = ctx.enter_context(tc.tile_pool(name="psB", bufs=2, space="PSUM"))
    psC = ctx.enter_context(tc.tile_pool(name="psC", bufs=2, space="PSUM"))

    def cload(name):
        shape, pdt = WSPEC[name]
        t = consts.tile(list(shape), pdt, tag=f"c_{name}")
        nc.sync.dma_start(out=t[:], in_=wext[name][:])
        return t

    w1 = cload("w1dr")          # [128, 4, 2, 128] f8
    w2 = cload("w2dr")          # [128, 4, 2, 128] f8
    wk = cload("wk8")           # [128, 128] f8
    g96 = cload("g96")          # [128, 96] f8
    qg = cload("qG")            # [128, 32] f32
    expd = cload("expand")      # [8, 128] f32
    idb = cload("ident96b")     # [96, 96] bf16
    id128 = cload("ident128b")  # [128, 128] bf16
    c1 = cload("c1")            # [128, 1] f32
    c2 = cload("c2")
    c3 = cload("c3")
    c4 = cload("c4")
    lamrow = cload("lamrow")    # [128, 8] f32

    def make_tile(it):
        p0g = it * NPAIR

        # pair-minor octet layout: x_sb[r, chunk, col, pair] so the DR rhs
        # flattens (col, pair) into one contiguous dim (3D AP requirement)
        x_sb = sbX.tile([128, 8, XW, 8], F8, tag="xsb")
        g0 = it * 8
        for qd in range(4):
            q0 = qd * 2
            nc.sync.dma_start(
                out=x_sb[:, q0:q0 + 2, :, :],
                in_=xe_ext[g0 + q0:g0 + q0 + 2].transpose([1, 0, 2, 3]))
        x_T = sbT.tile([128, NPAIR, 2, A], BF16, tag="xT")
        for hd in range(2):
            h0 = hd * (NPAIR // 2)
            nc.sync.dma_start(
                out=x_T[:, h0:h0 + NPAIR // 2, :, :],
                in_=xt_ext[p0g + h0:p0g + h0 + NPAIR // 2]
                    .transpose([1, 0, 2, 3]))

        h = sbH.tile([128, 8, E, 8], BF16, tag="h")
        h_T = sbH.tile([128, 8, E, 8], F8, tag="hT")
        k_T = sbK.tile([128, NPAIR, 112], BF16, tag="kT")
        k2 = sbK.tile([128, NPAIR, 112], BF16, tag="k2")
        qbk = sbQ.tile([128, NPAIR, 112], BF16, tag="qbk")
        q_acc = sb2.tile([128, 128], F32, tag="qacc")
        st = dict(it=it, k_T=k_T, k2=k2, qbk=qbk, q_acc=q_acc)

        xb = x_sb[0:120, 0:1, 0:1, 0:1].squeeze(3).squeeze(2).squeeze(1)
        hb = h_T[:, 0:1, 0:1, 0:1].squeeze(3).squeeze(2).squeeze(1)

        def conv1_chunk(c):
            ps = psA.tile([128, 1024], F32, tag="ps1")
            for ti in range(4):
                for hf in range(2):
                    rhs = apx(xb, [[8, 2], [1, 512]],
                              c * (XW * 8) + 16 * ti + 512 * hf)
                    nc.tensor.matmul(ps[:, 512 * hf:512 * (hf + 1)],
                                     w1[0:120, ti], rhs,
                                     start=(ti == 0), stop=(ti == 3),
                                     perf_mode=DR)
            th = sb2.tile([128, 1024], BF16, tag="th")
            nc.scalar.activation(th[:], ps[:], AF.Tanh, scale=1.0 / (2 * W1S))
            nc.vector.scalar_tensor_tensor(
                out=h[:, c].rearrange("p e q -> p (e q)"),
                in0=th[:], scalar=1.0, in1=ps[:], op0=OP.add, op1=OP.mult)

        def transpose_block(c):
            for hf in range(2):
                psx = psC.tile([128, 4, E], BF16, tag="psb")
                for i in range(4):
                    pr = 4 * hf + i
                    nc.tensor.transpose(
                        psx[:, i, :],
                        h[:, c, :, pr:pr + 1].squeeze(2), id128[:])
                dst = h_T[:, c, :, 4 * hf:4 * hf + 4].transpose([0, 2, 1])
                nc.scalar.copy(out=dst, in_=psx[:])

        def conv2_chunk(c):
            ps2 = psA.tile([128, 1024], F32, tag="ps1")
            for ti in range(4):
                for eo in range(2):
                    rhs = apx(hb, [[8, 2], [1, 448]],
                              c * (E * 8) + (62 * eo + 2 * ti) * 8)
                    nc.tensor.matmul(ps2[:, 512 * eo:512 * eo + 448],
                                     w2[:, ti], rhs,
                                     start=(ti == 0), stop=(ti == 3),
                                     perf_mode=DR)
            ps2_b = ps2[:, 0:1].squeeze(1)
            ps2v = apx(ps2_b, [[512, 2], [1, 448]], 0)
            th2 = sb2.tile([128, 896], BF16, tag="th2")
            nc.scalar.activation(th2[:], ps2v, AF.Tanh,
                                 scale=1.0 / (4 * W1S * W2S))
            h2s = sb2.tile([128, 896], BF16, tag="h2s")
            nc.vector.scalar_tensor_tensor(
                out=h2s[:], in0=th2[:], scalar=1.0, in1=ps2v,
                op0=OP.add, op1=OP.mult)
            # q accumulation: h2s cols are (eo, a, pair); q_acc cols are
            # sample-ordered (pair-major, eo minor) -> strided out view
            qa_b = q_acc[:, 0:1].squeeze(1)
            h2_b = h2s[:, 0:1].squeeze(1)
            nc.vector.reduce_sum(
                apx(qa_b, [[1, 2], [2, 8]], 16 * c),
                apx(h2_b, [[448, 2], [1, 8], [8, A]], 0),
                axis=AX.X)

        def kproj_block(c):
            for hf in range(2):
                b = 2 * c + hf
                psk = psB.tile([128, 512], F32, tag="psk")
                rhs = x_T[:, 4 * b:4 * b + 4, :, :].rearrange(
                    "p q c a -> p (q c a)")
                nc.tensor.matmul(psk[:, 0:448], wk[:], rhs,
                                 start=True, stop=True)
                dst = k_T[:, 4 * b:4 * b + 4, :].rearrange(
                    "p q r -> p (q r)")
                if hf == 0:
                    nc.scalar.copy(out=dst, in_=psk[:, 0:448])
                else:
                    nc.vector.tensor_copy(out=dst, in_=psk[:, 0:448])
            p0 = c * 8
            nc.gpsimd.tensor_tensor(
                out=k2[:, p0:p0 + 8, :], in0=k_T[:, p0:p0 + 8, :],
                in1=k_T[:, p0:p0 + 8, :], op=OP.mult)

        def step(s):
            if s < 8:
                conv1_chunk(s)
            if 1 <= s <= 8:
                transpose_block(s - 1)
            if s >= 2:
                conv2_chunk(s - 2)
            if s < 8:
                kproj_block(s)

        return step, st

    def make_tail(st):
        it = st["it"]
        k_T, k2, qbk, q_acc = st["k_T"], st["k2"], st["qbk"], st["q_acc"]
        sh = {}

        def p0():
            qpsA = psB.tile([128, 512], F32, tag="psk")
            nc.tensor.matmul(qpsA[0:8, 0:128], qg[:, 0:8], q_acc[:],
                             start=True, stop=True)           # muq
            q2 = sbQ.tile([128, 128], F32, tag="q2")
            nc.scalar.activation(q2[:], q_acc[:], AF.Square)
            nc.tensor.matmul(qpsA[0:8, 128:256], qg[:, 8:16], q2[:],
                             start=True, stop=True)           # sum q^2
            mq2 = sbQ.tile([128, 256], F32, tag="mq2")
            nc.vector.tensor_copy(out=mq2[0:8, :], in_=qpsA[0:8, 0:256])
            sh["mq2"] = mq2

        def p1():
            mq2 = sh["mq2"]
            vq = sbQ.tile([128, 128], F32, tag="vq")
            nc.vector.tensor_tensor(out=vq[0:8, :], in0=mq2[0:8, 0:128],
                                    in1=mq2[0:8, 0:128], op=OP.mult)
            nc.vector.scalar_tensor_tensor(
                out=vq[0:8, :], in0=mq2[0:8, 128:256], scalar=1.0 / HD,
                in1=vq[0:8, :], op0=OP.mult, op1=OP.subtract)
            nc.vector.tensor_scalar_add(vq[0:8, :], vq[0:8, :], EPS_Q)
            _newton_rsqrt(nc, sbQ, vq[0:8, :], [128, 128], "rq")
            sh["vq"] = vq

        def p2():
            mq2, vq = sh["mq2"], sh["vq"]
            qpsA = psB.tile([128, 512], F32, tag="psk")
            nc.tensor.matmul(qpsA[:, 0:128], expd[:], mq2[0:8, 0:128],
                             start=True, stop=True)
            nc.tensor.matmul(qpsA[:, 128:256], expd[:], vq[0:8, :],
                             start=True, stop=True)
            qhat = sbQ.tile([128, 128], F32, tag="qhat")
            nc.vector.tensor_tensor(out=qhat[:], in0=q_acc[:],
                                    in1=qpsA[:, 0:128], op=OP.subtract)
            nc.vector.tensor_tensor(out=qhat[:], in0=qhat[:],
                                    in1=qpsA[:, 128:256], op=OP.mult)
            qb = sbQ.tile([128, 128], F32, tag="qb")
            nc.vector.tensor_scalar(out=qb[:], in0=qhat[:], scalar1=c1[:],
                                    scalar2=c2[:], op0=OP.mult, op1=OP.add)
            sh["qhat"], sh["qb"] = qhat, qb

        def p3():
            qhat, qb = sh["qhat"], sh["qb"]
            qpsB = psB.tile([128, 512], F32, tag="psk")
            nc.tensor.matmul(qpsB[0:8, 0:128], qg[:, 16:24], qb[:],
                             start=True, stop=True)           # QB
            if HAS_QC:
                qa = sbQ.tile([128, 128], F32, tag="qa")
                nc.vector.tensor_scalar(out=qa[:], in0=qhat[:],
                                        scalar1=c3[:], scalar2=c4[:],
                                        op0=OP.mult, op1=OP.add)
                nc.tensor.matmul(qpsB[32:40, 128:256], qg[:, 24:32], qa[:],
                                 start=True, stop=True,
                                 tile_position=(0, 32))       # QC
            qcat = sbQ.tile([64, 128], BF16, tag="qcat")
            nc.vector.memset(qcat[:], 0.0)
            nc.vector.tensor_copy(out=qcat[0:8, :], in_=qpsB[0:8, 0:128])
            if HAS_QC:
                nc.vector.tensor_copy(out=qcat[32:40, :],
                                      in_=qpsB[32:40, 128:256])
            qps = psC.tile([128, 64], BF16, tag="psb")
            nc.tensor.transpose(qps[:, 0:64], qcat[0:64, :],
                                idb[0:64, 0:64])
            qsb = sb2.tile([128, 64], F32, tag="qsb")
            nc.vector.tensor_copy(out=qsb[:], in_=qps[:, 0:64])
            qbb = sbQ.tile([128, 128], BF16, tag="qbb")
            nc.gpsimd.tensor_copy(out=qbb[:], in_=qb[:])
            sh["qsb"], sh["qbb"] = qsb, qbb

        def qbk_piece(lo, hi):
            def f():
                qbb = sh["qbb"]
                for cki in range(lo, hi):
                    p0_ = cki * 8
                    nc.gpsimd.tensor_tensor(
                        out=qbk[:, p0_:p0_ + 8, :],
                        in0=k_T[:, p0_:p0_ + 8, :].rearrange(
                            "p q (c a) -> p q c a", c=2),
                        in1=qbb[:, 2 * p0_:2 * p0_ + 16]
                            .rearrange("p (q c) -> p q c", c=2).unsqueeze(3)
                            .to_broadcast((128, 8, 2, A)),
                        op=OP.mult)
            return f

        def stats_alloc():
            stats_sb = sb2.tile([128, NPAIR, 112], BF16, tag="statsb")
            sh["stats_sb"] = stats_sb

        def stats_piece(lo, hi):
            def f():
                stats_sb = sh["stats_sb"]
                for cki in range(lo, hi):
                    p0_ = cki * 4
                    pst = psB.tile([128, 512], F32, tag="psk")
                    nc.tensor.matmul(pst[0:32, 0:448], g96[:, 0:32],
                                     k_T[:, p0_:p0_ + 4, :],
                                     start=True, stop=True)
                    nc.tensor.matmul(pst[32:64, 0:448], g96[:, 32:64],
                                     k2[:, p0_:p0_ + 4, :],
                                     start=True, stop=True,
                                     tile_position=(0, 32))
                    nc.tensor.matmul(pst[64:96, 0:448], g96[:, 64:96],
                                     qbk[:, p0_:p0_ + 4, :],
                                     start=True, stop=True,
                                     tile_position=(0, 64))
                    dst = stats_sb[0:96, p0_:p0_ + 4, :].rearrange(
                        "p q r -> p (q r)")
                    if cki % 2 == 0:
                        nc.vector.tensor_copy(out=dst, in_=pst[0:96, 0:448])
                    else:
                        nc.scalar.copy(out=dst, in_=pst[0:96, 0:448])
            return f

        def statsB_piece(lo, hi):
            def f():
                if "statsB" not in sh:
                    statsB_t = sbQ.tile([128, A, 96], BF16, tag="statsB")
                    sh["statsB"] = statsB_t
                statsB = sh["statsB"]
                svb = sh["stats_sb"][:].rearrange("p q (c l) -> p (q c) l",
                                                  c=2)
                for li in range(lo, hi):
                    l0 = li * 7
                    pstb = psC.tile([128, 7, 96], BF16, tag="psb")
                    for j in range(7):
                        nc.tensor.transpose(pstb[:, j, :],
                                            svb[0:96, :, l0 + j],
                                            idb[0:96, 0:96])
                    if li % 2 == 0:
                        nc.vector.tensor_copy(out=statsB[:, l0:l0 + 7, :],
                                              in_=pstb[:])
                    else:
                        nc.scalar.copy(out=statsB[:, l0:l0 + 7, :],
                                       in_=pstb[:])
            return f

        def p_asm():
            statsB = sh["statsB"]
            qsb = sh["qsb"]
            muk = statsB[:, :, 0:8]
            sk2 = statsB[:, :, 32:40]
            QK = statsB[:, :, 64:72]
            vk = sbQ.tile([128, A, 8], F32, tag="vk")
            nc.vector.tensor_tensor(out=vk[:], in0=muk, in1=muk, op=OP.mult)
            nc.vector.scalar_tensor_tensor(out=vk[:], in0=sk2,
                                           scalar=1.0 / HD, in1=vk[:],
                                           op0=OP.mult, op1=OP.subtract)
            nc.vector.tensor_scalar_add(vk[:], vk[:], EPS_K)
            _newton_rsqrt(nc, sbQ, vk[:], [128, A, 8], "rk")
            s_sc = sbQ.tile([128, A, 8], F32, tag="ssc")
            QBb = qsb[:, 0:8].unsqueeze(1).to_broadcast((128, A, 8))
            QCb = qsb[:, 32:40].unsqueeze(1).to_broadcast((128, A, 8))
            nc.vector.tensor_tensor(out=s_sc[:], in0=muk, in1=QBb,
                                    op=OP.mult)
            nc.vector.tensor_tensor(out=s_sc[:], in0=QK, in1=s_sc[:],
                                    op=OP.subtract)
            nc.vector.tensor_tensor(out=s_sc[:], in0=s_sc[:], in1=vk[:],
                                    op=OP.mult)
            if HAS_QC:
                nc.vector.tensor_tensor(out=s_sc[:], in0=s_sc[:], in1=QCb,
                                        op=OP.add)
            sh["s_sc"] = s_sc

        def p_sm():
            s_sc = sh["s_sc"]
            # scores are bounded (|s| = O(4)): exp cannot overflow, so
            # skip the max-subtraction pass (softmax is shift-invariant).
            nc.scalar.activation(s_sc[:], s_sc[:], AF.Exp)
            z1 = sbQ.tile([128, 8], F32, tag="z1")
            nc.vector.reduce_sum(z1[:], s_sc[:].transpose([0, 2, 1]),
                                 axis=AX.X)
            rz1 = sbQ.tile([128, 8], F32, tag="rz1")
            nc.vector.reciprocal(rz1[:], z1[:])
            nc.vector.tensor_tensor(out=rz1[:], in0=rz1[:], in1=lamrow[:],
                                    op=OP.mult)
            nc.vector.tensor_tensor(
                out=s_sc[:], in0=s_sc[:],
                in1=rz1[:].unsqueeze(1).to_broadcast((128, A, 8)),
                op=OP.mult)
            dd = sbQ.tile([128, A, 4], F32, tag="dd")
            nc.vector.tensor_tensor(out=dd[:], in0=s_sc[:, :, 0:8:2],
                                    in1=s_sc[:, :, 1:8:2], op=OP.subtract)
            nc.scalar.activation(dd[:], dd[:], AF.Exp)
            z2 = sbQ.tile([128, 4], F32, tag="z2")
            nc.vector.reduce_sum(z2[:], dd[:].transpose([0, 2, 1]),
                                 axis=AX.X)
            rz2 = sbQ.tile([128, 4], F32, tag="rz2")
            nc.vector.reciprocal(rz2[:], z2[:])
            nc.vector.tensor_scalar_mul(rz2[:], rz2[:], 1.0 / H)
            nc.vector.tensor_tensor(
                out=dd[:], in0=dd[:],
                in1=rz2[:].unsqueeze(1).to_broadcast((128, A, 4)),
                op=OP.mult)
            ot = sbQ.tile([128, A], F32, tag="ot")
            nc.vector.reduce_sum(ot[:], dd[:], axis=AX.X)
            nc.sync.dma_start(out=out_ext[it * BT:(it + 1) * BT, :],
                              in_=ot[:])

        def p_stats_alloc_and_first():
            stats_alloc()
            stats_piece(0, 4)()

        early = [p0, p1, p2, p3,
                 qbk_piece(0, 4), qbk_piece(4, 8),
                 p_stats_alloc_and_first,
                 stats_piece(4, 9), stats_piece(9, 13), stats_piece(13, 16)]
        late = [statsB_piece(0, 4), statsB_piece(4, 8), p_asm, p_sm]
        return early, late

    E_SLOTS = [0, 1, 2, 3, 4, 5, 6, 7, 8, 9]
    L_SLOTS = [0, 1, 2, 4]
    pend_E, pend_L = [], []
    for it_ in range(NT * reps):
        step, st = make_tile(it_ % NT)
        sched = {}
        for p, s in zip(pend_L, L_SLOTS):
            sched.setdefault(s, []).append(p)
        for p, s in zip(pend_E, E_SLOTS):
            sched.setdefault(s, []).append(p)
        for s in range(10):
            step(s)
            for piece in sched.get(s, []):
                piece()
        new_E, new_L = make_tail(st)
        pend_L = pend_E_L if (pend_E_L := getattr(make_tile, "_carryL", None)) else []
        make_tile._carryL = new_L
        pend_E = new_E
    for piece in pend_L:
        piece()
    for piece in pend_E:
        piece()
    for piece in getattr(make_tile, "_carryL", []):
        piece()


def build_nc(reps=1):
    nc = bacc.Bacc(target_bir_lowering=False, trn_type="TRN2")
    xe_ext = nc.declare_dram_parameter("x", [NB // 16, 128, XW, 8], F8,
                                       isOutput=False)
    xt_ext = nc.declare_dram_parameter("xt", [NB // 2, 128, 2, A], BF16,
                                       isOutput=False)
    out_ext = nc.declare_dram_parameter("out", [NB, A], F32, isOutput=True)
    wext = {}
    for name, (shape, dt) in WSPEC.items():
        wext[name] = nc.declare_dram_parameter(name, list(shape), dt,
                                               isOutput=False)
    with tile.TileContext(nc) as tc:
        with ExitStack() as ctx:
            build_tile_kernel(ctx, tc, xe_ext, xt_ext, out_ext, wext,
                              reps=reps)
    nc.compile()
    return nc


def prepare_weights(w_emb, b_emb, w_atlas, b_atlas, w_k, qn_w, qn_b, kn_w,
                    kn_b, lambda_q1, lambda_k1, lambda_q2, lambda_k2):
    import ml_dtypes
    bf = ml_dtypes.bfloat16
    e4 = ml_dtypes.float8_e4m3
    f32 = np.float32

    assert np.allclose(b_atlas, 0.0), "kernel assumes b_atlas == 0"
    assert np.allclose(b_emb, 0.0), "kernel assumes b_emb == 0"

    def q8(v):
        return np.clip(v, -240.0, 240.0).astype(e4)

    # conv1 lhsT [120, (ti,j)=tap, 128]: contract rows match pair-image rows
    # (0-55 even, 64-119 odd); out cols match h col windows (3:59 / 65:121)
    w1dr = np.zeros((128, 4, 2, 128), f32)
    for t in range(7):
        blk = W1S * np.transpose(w_emb[:, :, t]).astype(f32)
        w1dr[0:56, t // 2, t % 2, 3:59] = blk
        w1dr[64:120, t // 2, t % 2, 65:121] = blk

    # conv2 lhsT [e_in, tap, e_out]
    w2dr = np.zeros((128, 4, 2, 128), f32)
    for t in range(7):
        w2dr[:, t // 2, t % 2, :] = W2S * np.transpose(
            w_atlas[:, :, t]).astype(f32)

    wk8 = q8(WKS * np.transpose(w_k))

    G = np.zeros((128, 8), f32)
    for o in range(128):
        G[o, o // HD] = 1.0
    Z24 = np.zeros((128, 24), f32)
    g96 = q8(np.concatenate([G / HD, Z24, G, Z24, G, Z24], axis=1))

    d_idx = np.arange(E) % HD
    bkd = kn_b[d_idx].astype(f32)
    qG = np.ascontiguousarray(
        np.concatenate([G / HD, G, G, G * bkd[:, None]], axis=1)).astype(f32)

    expand = np.ascontiguousarray(G.T).astype(f32)            # [8, 128]
    ident96b = np.eye(96, dtype=bf)
    ident128b = np.eye(128, dtype=bf)

    c1 = (SCALING * qn_w[d_idx] * kn_w[d_idx]).astype(f32).reshape(128, 1)
    c2 = (SCALING * qn_b[d_idx] * kn_w[d_idx]).astype(f32).reshape(128, 1)
    c3 = (SCALING * qn_w[d_idx]).astype(f32).reshape(128, 1)
    c4 = (SCALING * qn_b[d_idx]).astype(f32).reshape(128, 1)

    lam = float(np.exp(np.sum(lambda_q1 * lambda_k1))
                - np.exp(np.sum(lambda_q2 * lambda_k2)) + LAMBDA_INIT)
    lamrow = np.tile(np.array([1.0, lam] * 4, f32), (128, 1))

    global HAS_QC
    HAS_QC = bool(np.any(kn_b != 0.0))

    wdict = dict(w1dr=q8(w1dr), w2dr=q8(w2dr), wk8=wk8, g96=g96, qG=qG,
                 expand=expand, ident96b=ident96b, ident128b=ident128b,
                 c1=c1, c2=c2, c3=c3, c4=c4, lamrow=lamrow)
    WSPEC.clear()
    dtmap = {np.dtype(np.float32): F32, np.dtype(bf): BF16,
             np.dtype(e4): F8}
    for k, v in wdict.items():
        WSPEC[k] = (v.shape, dtmap[v.dtype])
    return wdict


def pack_x(x):
    """x [N, 56, 128] f32 -> (xe [N//2, 128, 160] f8, xt [N//2, 128, 2, 56])."""
    import ml_dtypes
    e4 = ml_dtypes.float8_e4m3
    xf = np.asarray(x, np.float32)
    x8 = np.clip(xf, -240, 240).astype(e4)
    n = x8.shape[0]
    xe = np.zeros((n // 16, 128, XW, 8), e4)
    ev = x8[0::2].reshape(n // 16, 8, A, E)    # [g, pair, a, e]
    od = x8[1::2].reshape(n // 16, 8, A, E)
    xe[:, 0:56, 3:3 + E, :] = ev.transpose(0, 2, 3, 1)
    xe[:, 64:120, 3:3 + E, :] = od.transpose(0, 2, 3, 1)
    bfd = ml_dtypes.bfloat16
    xb = xf.astype(bfd)
    xt = np.zeros((n // 2, 128, 2, A), bfd)
    xt[:, :, 0, :] = xb[0::2].transpose(0, 2, 1)
    xt[:, :, 1, :] = xb[1::2].transpose(0, 2, 1)
    return xe, xt


_CACHED = {}


def kernel(**inputs):
    xe, xt = pack_x(inputs["x"])
    wdict = prepare_weights(
        **{k: np.asarray(v, np.float32) for k, v in inputs.items()
           if k != "x"})
    if "nc" not in _CACHED:
        _CACHED["nc"] = build_nc()
    nc = _CACHED["nc"]
    nbp = NB // 2
    in_maps = []
    for c in range(NCORES):
        m = {"x": np.ascontiguousarray(xe[c * nbp:(c + 1) * nbp]),
             "xt": np.ascontiguousarray(xt[c * nbp:(c + 1) * nbp])}
        m.update(wdict)
        in_maps.append(m)
    res = run_bass_kernel_spmd(nc, in_maps, core_ids=list(range(NCORES)))
    return np.concatenate([np.asarray(r["out"]) for r in res.results], axis=0)


if __name__ == "__main__":
    import reference
    inputs = {k: np.asarray(v) for k, v in reference.setup_inputs().items()}
    got = kernel(**inputs)
    exp = np.asarray(reference.reference(**inputs))
    err = np.abs(got - exp).max() / np.abs(exp).max()
    print("rel err:", err)


# revision 27
# speedup vs baseline: 1.1938x; 1.1938x over previous
"""Trainium2 Bass kernel for nn_AtlasMultiDiffAttn (8-core data-parallel).

Self-contained: hardcodes shapes (x [8192,56,128] f32 -> out [8192,56] f32).
Per core: 1024 samples, 8 tiles of BT=128 samples (64 even/odd sample pairs).

v2: fp8 (e4m3, TRN +-240) DoubleRow convs.
  - conv1/conv2 matmuls run fp8 DoubleRow (2 taps per instruction, Ko step=1
    overlapping-window rhs - verified on HW), 0.5 cyc/row.
  - x arrives in two host layouts: pair-image xe [pair, row, 160] fp8 for
    conv1 (rows 0-55 even / 64-119 odd sample channels, col c = x[., c-3]),
    and xt [pair, e, 2, 56] fp8 for k-proj (no on-device x transposes).
  - scales: w1 x16, w2 x16, wk x8 keep fp8 weights out of subnormals;
    h = 32*silu (<240), k_T = 8*k; layernorms absorb scales via
    EPS_Q = (1024*56)^2*EPS and EPS_K = 64*EPS.
  - silu via tanh (exp_and_others act table; no table switches):
    2silu(y) = (tanh(y/2)+1)*y with y held scaled in PSUM.
  - h_T via PE transposes (fp8); stats/scores as in v1 but fp8/bf16 operands.
"""
from contextlib import ExitStack

import numpy as np

import concourse.bass as bass
import concourse.tile as tile
from concourse import bacc, mybir
from concourse.bass_utils import run_bass_kernel_spmd

F32 = mybir.dt.float32
BF16 = mybir.dt.bfloat16
F8 = mybir.dt.float8e4
I32 = mybir.dt.int32
AF = mybir.ActivationFunctionType
OP = mybir.AluOpType
AX = mybir.AxisListType
DR = mybir.MatmulPerfMode.DoubleRow

B, A, E = 8192, 56, 128
H, HD = 4, 16
LAMBDA_INIT = 0.7
EPS = 1e-5
SCALING = HD ** -0.5

W1S = 16.0                # conv1 weight quant scale
W2S = 16.0                # conv2 weight quant scale
WKS = 8.0                 # k-proj weight quant scale
QC_SCALE = 4.0 * W1S * W2S * A   # q_acc = QC_SCALE * q_true
EPS_Q = QC_SCALE * QC_SCALE * EPS
EPS_K = WKS * WKS * EPS

NCORES = 8
NB = B // NCORES          # 1024 samples per core
BT = 128                  # samples per tile
NT = NB // BT             # 8 tiles
NPAIR = BT // 2           # 64
XW = 136                  # padded conv1 col count (cols 3..130 = x)
MAGIC = 0x5F3759DF

WSPEC = {}
HAS_QC = True


def _newton_rsqrt(nc, pool, v_ap, shape, tag):
    """v_ap <- rsqrt(v_ap): magic init + 1 Newton iteration (~0.2% rel)."""
    y = pool.tile(list(shape), F32, tag=f"nwy_{tag}")
    t = pool.tile(list(shape), F32, tag=f"nwt_{tag}")
    npart = v_ap.shape[0]
    ya, ta = y[0:npart], t[0:npart]
    nc.vector.tensor_scalar(out=ya.bitcast(I32), in0=v_ap.bitcast(I32),
                            scalar1=1, scalar2=None,
                            op0=OP.logical_shift_right)
    nc.vector.tensor_scalar(out=ya.bitcast(I32), in0=ya.bitcast(I32),
                            scalar1=-1, scalar2=MAGIC,
                            op0=OP.mult, op1=OP.add)
    nc.vector.tensor_tensor(out=ta, in0=ya, in1=ya, op=OP.mult)
    nc.vector.tensor_tensor(out=ta, in0=ta, in1=v_ap, op=OP.mult)
    nc.vector.tensor_scalar(out=ta, in0=ta, scalar1=-0.5, scalar2=1.5,
                            op0=OP.mult, op1=OP.add)
    nc.vector.tensor_tensor(out=v_ap, in0=ya, in1=ta, op=OP.mult)


def apx(base, insert, offset_add):
    """Append (stride,count) dims to an AP and bump its offset."""
    return bass.AP(tensor=base.tensor, offset=base.offset + offset_add,
                   ap=list(base.ap) + [list(d) for d in insert])


def build_tile_kernel(ctx, tc, xe_ext, xt_ext, out_ext, wext, reps=1):
    nc = tc.nc

    consts = ctx.enter_context(tc.tile_pool(name="consts", bufs=1))
    sbX = ctx.enter_context(tc.tile_pool(name="sbX", bufs=2))
    sbT = ctx.enter_context(tc.tile_pool(name="sbT", bufs=2))
    sbH = ctx.enter_context(tc.tile_pool(name="sbH", bufs=1))
    sbK = ctx.enter_context(tc.tile_pool(name="sbK", bufs=2))
    sbS = ctx.enter_context(tc.tile_pool(name="sbS", bufs=2))
    sb2 = ctx.enter_context(tc.tile_pool(name="sb2", bufs=2))
    sbQ = ctx.enter_context(tc.tile_pool(name="sbQ", bufs=1))
    psA = ctx.enter_context(tc.tile_pool(name="psA", bufs=2, space="PSUM"))
    psB = ctx.enter_context(tc.tile_pool(name="psB", bufs=2, space="PSUM"))
    psC = ctx.enter_context(tc.tile_pool(name="psC", bufs=2, space="PSUM"))

    def cload(name):
        shape, pdt = WSPEC[name]
        t = consts.tile(list(shape), pdt, tag=f"c_{name}")
        nc.sync.dma_start(out=t[:], in_=wext[name][:])
        return t

    w1 = cload("w1dr")          # [128, 4, 2, 128] f8
    w2 = cload("w2dr")          # [128, 4, 2, 128] f8
    wk = cload("wk8")           # [128, 128] f8
    g96 = cload("g96")          # [128, 96] f8
    qg = cload("qG")            # [128, 32] f32
    expd = cload("expand")      # [8, 128] f32
    idb = cload("ident96b")     # [96, 96] bf16
    id128 = cload("ident128b")  # [128, 128] bf16
    c1 = cload("c1")            # [128, 1] f32
    c2 = cload("c2")
    c3 = cload("c3")
    c4 = cload("c4")
    lamrow = cload("lamrow")    # [128, 8] f32

    def make_tile(it):
        p0g = it * NPAIR

        # pair-minor octet layout: x_sb[r, chunk, col, pair] so the DR rhs
        # flattens (col, pair) into one contiguous dim (3D AP requirement)
        x_sb = sbX.tile([128, 8, XW, 8], F8, tag="xsb")
        g0 = it * 8
        for qd in range(4):
            q0 = qd * 2
            nc.sync.dma_start(
                out=x_sb[:, q0:q0 + 2, :, :],
                in_=xe_ext[g0 + q0:g0 + q0 + 2].transpose([1, 0, 2, 3]))
        x_T = sbT.tile([128, NPAIR, 2, A], BF16, tag="xT")
        for hd in range(2):
            h0 = hd * (NPAIR // 2)
            nc.sync.dma_start(
                out=x_T[:, h0:h0 + NPAIR // 2, :, :],
                in_=xt_ext[p0g + h0:p0g + h0 + NPAIR // 2]
                    .transpose([1, 0, 2, 3]))

        h = sbH.tile([128, 8, E, 8], BF16, tag="h")
        h_T = sbH.tile([128, 8, E, 8], F8, tag="hT")
        k_T = sbK.tile([128, NPAIR, 112], BF16, tag="kT")
        k2 = sbK.tile([128, NPAIR, 112], BF16, tag="k2")
        qbk = sbQ.tile([128, NPAIR, 112], BF16, tag="qbk")
        q_acc = sb2.tile([128, 128], F32, tag="qacc")
        st = dict(it=it, k_T=k_T, k2=k2, qbk=qbk, q_acc=q_acc)

        xb = x_sb[0:120, 0:1, 0:1, 0:1].squeeze(3).squeeze(2).squeeze(1)
        hb = h_T[:, 0:1, 0:1, 0:1].squeeze(3).squeeze(2).squeeze(1)

        def conv1_chunk(c):
            ps = psA.tile([128, 1024], F32, tag="ps1")
            for ti in range(4):
                for hf in range(2):
                    rhs = apx(xb, [[8, 2], [1, 512]],
                              c * (XW * 8) + 16 * ti + 512 * hf)
                    nc.tensor.matmul(ps[:, 512 * hf:512 * (hf + 1)],
                                     w1[0:120, ti], rhs,
                                     start=(ti == 0), stop=(ti == 3),
                                     perf_mode=DR)
            th = sb2.tile([128, 1024], BF16, tag="th")
            nc.scalar.activation(th[:], ps[:], AF.Tanh, scale=1.0 / (2 * W1S))
            nc.vector.scalar_tensor_tensor(
                out=h[:, c].rearrange("p e q -> p (e q)"),
                in0=th[:], scalar=1.0, in1=ps[:], op0=OP.add, op1=OP.mult)

        def transpose_block(c):
            for hf in range(2):
                psx = psC.tile([128, 4, E], BF16, tag="psb")
                for i in range(4):
                    pr = 4 * hf + i
                    nc.tensor.transpose(
                        psx[:, i, :],
                        h[:, c, :, pr:pr + 1].squeeze(2), id128[:])
                dst = h_T[:, c, :, 4 * hf:4 * hf + 4].transpose([0, 2, 1])
                nc.scalar.copy(out=dst, in_=psx[:])

        def conv2_chunk(c):
            ps2 = psA.tile([128, 1024], F32, tag="ps1")
            for ti in range(4):
                for eo in range(2):
                    rhs = apx(hb, [[8, 2], [1, 448]],
                              c * (E * 8) + (62 * eo + 2 * ti) * 8)
                    nc.tensor.matmul(ps2[:, 512 * eo:512 * eo + 448],
                                     w2[:, ti], rhs,
                                     start=(ti == 0), stop=(ti == 3),
                                     perf_mode=DR)
            ps2_b = ps2[:, 0:1].squeeze(1)
            ps2v = apx(ps2_b, [[512, 2], [1, 448]], 0)
            th2 = sb2.tile([128, 896], BF16, tag="th2")
            nc.scalar.activation(th2[:], ps2v, AF.Tanh,
                                 scale=1.0 / (4 * W1S * W2S))
            h2s = sb2.tile([128, 896], BF16, tag="h2s")
            nc.vector.scalar_tensor_tensor(
                out=h2s[:], in0=th2[:], scalar=1.0, in1=ps2v,
                op0=OP.add, op1=OP.mult)
            # q accumulation: h2s cols are (eo, a, pair); q_acc cols are
            # sample-ordered (pair-major, eo minor) -> strided out view
            qa_b = q_acc[:, 0:1].squeeze(1)
            h2_b = h2s[:, 0:1].squeeze(1)
            nc.vector.reduce_sum(
                apx(qa_b, [[1, 2], [2, 8]], 16 * c),
                apx(h2_b, [[448, 2], [1, 8], [8, A]], 0),
                axis=AX.X)

        def kproj_block(c):
            for hf in range(2):
                b = 2 * c + hf
                psk = psB.tile([128, 512], F32, tag="psk")
                rhs = x_T[:, 4 * b:4 * b + 4, :, :].rearrange(
                    "p q c a -> p (q c a)")
                nc.tensor.matmul(psk[:, 0:448], wk[:], rhs,
                                 start=True, stop=True)
                dst = k_T[:, 4 * b:4 * b + 4, :].rearrange(
                    "p q r -> p (q r)")
                nc.scalar.copy(out=dst, in_=psk[:, 0:448])
            p0 = c * 8
            nc.gpsimd.tensor_tensor(
                out=k2[:, p0:p0 + 8, :], in0=k_T[:, p0:p0 + 8, :],
                in1=k_T[:, p0:p0 + 8, :], op=OP.mult)

        def step(s):
            if s < 8:
                conv1_chunk(s)
            if 1 <= s <= 8:
                transpose_block(s - 1)
            if s >= 2:
                conv2_chunk(s - 2)
            if s < 8:
                kproj_block(s)

        return step, st

    def make_tail(st):
        it = st["it"]
        k_T, k2, qbk, q_acc = st["k_T"], st["k2"], st["qbk"], st["q_acc"]
        sh = {}

        def p0():
            qpsA = psB.tile([128, 512], F32, tag="psk")
            nc.tensor.matmul(qpsA[0:8, 0:128], qg[:, 0:8], q_acc[:],
                             start=True, stop=True)           # muq
            q2 = sbQ.tile([128, 128], F32, tag="q2")
            nc.scalar.activation(q2[:], q_acc[:], AF.Square)
            nc.tensor.matmul(qpsA[0:8, 128:256], qg[:, 8:16], q2[:],
                             start=True, stop=True)           # sum q^2
            mq2 = sbQ.tile([128, 256], F32, tag="mq2")
            nc.vector.tensor_copy(out=mq2[0:8, :], in_=qpsA[0:8, 0:256])
            sh["mq2"] = mq2

        def p1():
            mq2 = sh["mq2"]
            vq = sbQ.tile([128, 128], F32, tag="vq")
            nc.vector.tensor_tensor(out=vq[0:8, :], in0=mq2[0:8, 0:128],
                                    in1=mq2[0:8, 0:128], op=OP.mult)
            nc.vector.scalar_tensor_tensor(
                out=vq[0:8, :], in0=mq2[0:8, 128:256], scalar=1.0 / HD,
                in1=vq[0:8, :], op0=OP.mult, op1=OP.subtract)
            nc.vector.tensor_scalar_add(vq[0:8, :], vq[0:8, :], EPS_Q)
            _newton_rsqrt(nc, sbQ, vq[0:8, :], [128, 128], "rq")
            sh["vq"] = vq

        def p2():
            mq2, vq = sh["mq2"], sh["vq"]
            qpsA = psB.tile([128, 512], F32, tag="psk")
            nc.tensor.matmul(qpsA[:, 0:128], expd[:], mq2[0:8, 0:128],
                             start=True, stop=True)
            nc.tensor.matmul(qpsA[:, 128:256], expd[:], vq[0:8, :],
                             start=True, stop=True)
            qhat = sbQ.tile([128, 128], F32, tag="qhat")
            nc.vector.tensor_tensor(out=qhat[:], in0=q_acc[:],
                                    in1=qpsA[:, 0:128], op=OP.subtract)
            nc.vector.tensor_tensor(out=qhat[:], in0=qhat[:],
                                    in1=qpsA[:, 128:256], op=OP.mult)
            qb = sbQ.tile([128, 128], F32, tag="qb")
            nc.vector.tensor_scalar(out=qb[:], in0=qhat[:], scalar1=c1[:],
                                    scalar2=c2[:], op0=OP.mult, op1=OP.add)
            sh["qhat"], sh["qb"] = qhat, qb

        def p3():
            qhat, qb = sh["qhat"], sh["qb"]
            qpsB = psB.tile([128, 512], F32, tag="psk")
            nc.tensor.matmul(qpsB[0:8, 0:128], qg[:, 16:24], qb[:],
                             start=True, stop=True)           # QB
            if HAS_QC:
                qa = sbQ.tile([128, 128], F32, tag="qa")
                nc.vector.tensor_scalar(out=qa[:], in0=qhat[:],
                                        scalar1=c3[:], scalar2=c4[:],
                                        op0=OP.mult, op1=OP.add)
                nc.tensor.matmul(qpsB[32:40, 128:256], qg[:, 24:32], qa[:],
                                 start=True, stop=True,
                                 tile_position=(0, 32))       # QC
            qcat = sbQ.tile([64, 128], BF16, tag="qcat")
            nc.vector.memset(qcat[:], 0.0)
            nc.vector.tensor_copy(out=qcat[0:8, :], in_=qpsB[0:8, 0:128])
            if HAS_QC:
                nc.vector.tensor_copy(out=qcat[32:40, :],
                                      in_=qpsB[32:40, 128:256])
            qps = psC.tile([128, 64], BF16, tag="psb")
            nc.tensor.transpose(qps[:, 0:64], qcat[0:64, :],
                                idb[0:64, 0:64])
            qsb = sb2.tile([128, 64], F32, tag="qsb")
            nc.vector.tensor_copy(out=qsb[:], in_=qps[:, 0:64])
            qbb = sbQ.tile([128, 128], BF16, tag="qbb")
            nc.gpsimd.tensor_copy(out=qbb[:], in_=qb[:])
            sh["qsb"], sh["qbb"] = qsb, qbb

        def qbk_piece(lo, hi):
            def f():
                qbb = sh["qbb"]
                for cki in range(lo, hi):
                    p0_ = cki * 8
                    nc.gpsimd.tensor_tensor(
                        out=qbk[:, p0_:p0_ + 8, :],
                        in0=k_T[:, p0_:p0_ + 8, :].rearrange(
                            "p q (c a) -> p q c a", c=2),
                        in1=qbb[:, 2 * p0_:2 * p0_ + 16]
                            .rearrange("p (q c) -> p q c", c=2).unsqueeze(3)
                            .to_broadcast((128, 8, 2, A)),
                        op=OP.mult)
            return f

        def stats_alloc():
            stats_sb = sb2.tile([128, NPAIR, 112], BF16, tag="statsb")
            sh["stats_sb"] = stats_sb

        def stats_piece(lo, hi):
            def f():
                stats_sb = sh["stats_sb"]
                for cki in range(lo, hi):
                    p0_ = cki * 4
                    pst = psB.tile([128, 512], F32, tag="psk")
                    nc.tensor.matmul(pst[0:32, 0:448], g96[:, 0:32],
                                     k_T[:, p0_:p0_ + 4, :],
                                     start=True, stop=True)
                    nc.tensor.matmul(pst[32:64, 0:448], g96[:, 32:64],
                                     k2[:, p0_:p0_ + 4, :],
                                     start=True, stop=True,
                                     tile_position=(0, 32))
                    nc.tensor.matmul(pst[64:96, 0:448], g96[:, 64:96],
                                     qbk[:, p0_:p0_ + 4, :],
                                     start=True, stop=True,
                                     tile_position=(0, 64))
                    dst = stats_sb[0:96, p0_:p0_ + 4, :].rearrange(
                        "p q r -> p (q r)")
                    if cki % 2 == 0:
                        nc.vector.tensor_copy(out=dst, in_=pst[0:96, 0:448])
                    else:
                        nc.scalar.copy(out=dst, in_=pst[0:96, 0:448])
            return f

        def statsB_piece(lo, hi):
            def f():
                if "statsB" not in sh:
                    statsB_t = sbQ.tile([128, A, 96], BF16, tag="statsB")
                    sh["statsB"] = statsB_t
                statsB = sh["statsB"]
                svb = sh["stats_sb"][:].rearrange("p q (c l) -> p (q c) l",
                                                  c=2)
                for li in range(lo, hi):
                    l0 = li * 7
                    pstb = psC.tile([128, 7, 96], BF16, tag="psb")
                    for j in range(7):
                        nc.tensor.transpose(pstb[:, j, :],
                                            svb[0:96, :, l0 + j],
                                            idb[0:96, 0:96])
                    if li % 2 == 0:
                        nc.vector.tensor_copy(out=statsB[:, l0:l0 + 7, :],
                                              in_=pstb[:])
                    else:
                        nc.scalar.copy(out=statsB[:, l0:l0 + 7, :],
                                       in_=pstb[:])
            return f

        def p_asm():
            statsB = sh["statsB"]
            qsb = sh["qsb"]
            muk = statsB[:, :, 0:8]
            sk2 = statsB[:, :, 32:40]
            QK = statsB[:, :, 64:72]
            vk = sbQ.tile([128, A, 8], F32, tag="vk")
            nc.vector.tensor_tensor(out=vk[:], in0=muk, in1=muk, op=OP.mult)
            nc.vector.scalar_tensor_tensor(out=vk[:], in0=sk2,
                                           scalar=1.0 / HD, in1=vk[:],
                                           op0=OP.mult, op1=OP.subtract)
            nc.vector.tensor_scalar_add(vk[:], vk[:], EPS_K)
            _newton_rsqrt(nc, sbQ, vk[:], [128, A, 8], "rk")
            s_sc = sbQ.tile([128, A, 8], F32, tag="ssc")
            QBb = qsb[:, 0:8].unsqueeze(1).to_broadcast((128, A, 8))
            QCb = qsb[:, 32:40].unsqueeze(1).to_broadcast((128, A, 8))
            nc.vector.tensor_tensor(out=s_sc[:], in0=muk, in1=QBb,
                                    op=OP.mult)
            nc.vector.tensor_tensor(out=s_sc[:], in0=QK, in1=s_sc[:],
                                    op=OP.subtract)
            nc.vector.tensor_tensor(out=s_sc[:], in0=s_sc[:], in1=vk[:],
                                    op=OP.mult)
            if HAS_QC:
                nc.vector.tensor_tensor(out=s_sc[:], in0=s_sc[:], in1=QCb,
                                        op=OP.add)
            sh["s_sc"] = s_sc

        def p_sm():
            s_sc = sh["s_sc"]
            # scores are bounded (|s| = O(4)): exp cannot overflow, so
            # skip the max-subtraction pass (softmax is shift-invariant).
            nc.scalar.activation(s_sc[:], s_sc[:], AF.Exp)
            z1 = sbQ.tile([128, 8], F32, tag="z1")
            nc.vector.reduce_sum(z1[:], s_sc[:].transpose([0, 2, 1]),
                                 axis=AX.X)
            rz1 = sbQ.tile([128, 8], F32, tag="rz1")
            nc.vector.reciprocal(rz1[:], z1[:])
            nc.vector.tensor_tensor(out=rz1[:], in0=rz1[:], in1=lamrow[:],
                                    op=OP.mult)
            nc.vector.tensor_tensor(
                out=s_sc[:], in0=s_sc[:],
                in1=rz1[:].unsqueeze(1).to_broadcast((128, A, 8)),
                op=OP.mult)
            dd = sbQ.tile([128, A, 4], F32, tag="dd")
            nc.vector.tensor_tensor(out=dd[:], in0=s_sc[:, :, 0:8:2],
                                    in1=s_sc[:, :, 1:8:2], op=OP.subtract)
            nc.scalar.activation(dd[:], dd[:], AF.Exp)
            z2 = sbQ.tile([128, 4], F32, tag="z2")
            nc.vector.reduce_sum(z2[:], dd[:].transpose([0, 2, 1]),
                                 axis=AX.X)
            rz2 = sbQ.tile([128, 4], F32, tag="rz2")
            nc.vector.reciprocal(rz2[:], z2[:])
            nc.vector.tensor_scalar_mul(rz2[:], rz2[:], 1.0 / H)
            nc.vector.tensor_tensor(
                out=dd[:], in0=dd[:],
                in1=rz2[:].unsqueeze(1).to_broadcast((128, A, 4)),
                op=OP.mult)
            ot = sbQ.tile([128, A], F32, tag="ot")
            nc.vector.reduce_sum(ot[:], dd[:], axis=AX.X)
            nc.sync.dma_start(out=out_ext[it * BT:(it + 1) * BT, :],
                              in_=ot[:])

        def p_stats_alloc_and_first():
            stats_alloc()
            stats_piece(0, 4)()

        early = [p0, p1, p2, p3,
                 qbk_piece(0, 4), qbk_piece(4, 8),
                 p_stats_alloc_and_first,
                 stats_piece(4, 9), stats_piece(9, 13), stats_piece(13, 16)]
        late = [statsB_piece(0, 4), statsB_piece(4, 8), p_asm, p_sm]
        return early, late

    E_SLOTS = [0, 1, 2, 3, 4, 5, 6, 7, 8, 9]
    L_SLOTS = [0, 1, 2, 4]
    pend_E, pend_L = [], []
    for it_ in range(NT * reps):
        step, st = make_tile(it_ % NT)
        sched = {}
        for p, s in zip(pend_L, L_SLOTS):
            sched.setdefault(s, []).append(p)
        for p, s in zip(pend_E, E_SLOTS):
            sched.setdefault(s, []).append(p)
        for s in range(10):
            step(s)
            for piece in sched.get(s, []):
                piece()
        new_E, new_L = make_tail(st)
        pend_L = pend_E_L if (pend_E_L := getattr(make_tile, "_carryL", None)) else []
        make_tile._carryL = new_L
        pend_E = new_E
    for piece in pend_L:
        piece()
    for piece in pend_E:
        piece()
    for piece in getattr(make_tile, "_carryL", []):
        piece()


def build_nc(reps=1):
    nc = bacc.Bacc(target_bir_lowering=False, trn_type="TRN2")
    xe_ext = nc.declare_dram_parameter("x", [NB // 16, 128, XW, 8], F8,
                                       isOutput=False)
    xt_ext = nc.declare_dram_parameter("xt", [NB // 2, 128, 2, A], BF16,
                                       isOutput=False)
    out_ext = nc.declare_dram_parameter("out", [NB, A], F32, isOutput=True)
    wext = {}
    for name, (shape, dt) in WSPEC.items():
        wext[name] = nc.declare_dram_parameter(name, list(shape), dt,
                                               isOutput=False)
    with tile.TileContext(nc) as tc:
        with ExitStack() as ctx:
            build_tile_kernel(ctx, tc, xe_ext, xt_ext, out_ext, wext,
                              reps=reps)
    nc.compile()
    return nc


def prepare_weights(w_emb, b_emb, w_atlas, b_atlas, w_k, qn_w, qn_b, kn_w,
                    kn_b, lambda_q1, lambda_k1, lambda_q2, lambda_k2):
    import ml_dtypes
    bf = ml_dtypes.bfloat16
    e4 = ml_dtypes.float8_e4m3
    f32 = np.float32

    assert np.allclose(b_atlas, 0.0), "kernel assumes b_atlas == 0"
    assert np.allclose(b_emb, 0.0), "kernel assumes b_emb == 0"

    def q8(v):
        return np.clip(v, -240.0, 240.0).astype(e4)

    # conv1 lhsT [120, (ti,j)=tap, 128]: contract rows match pair-image rows
    # (0-55 even, 64-119 odd); out cols match h col windows (3:59 / 65:121)
    w1dr = np.zeros((128, 4, 2, 128), f32)
    for t in range(7):
        blk = W1S * np.transpose(w_emb[:, :, t]).astype(f32)
        w1dr[0:56, t // 2, t % 2, 3:59] = blk
        w1dr[64:120, t // 2, t % 2, 65:121] = blk

    # conv2 lhsT [e_in, tap, e_out]
    w2dr = np.zeros((128, 4, 2, 128), f32)
    for t in range(7):
        w2dr[:, t // 2, t % 2, :] = W2S * np.transpose(
            w_atlas[:, :, t]).astype(f32)

    wk8 = q8(WKS * np.transpose(w_k))

    G = np.zeros((128, 8), f32)
    for o in range(128):
        G[o, o // HD] = 1.0
    Z24 = np.zeros((128, 24), f32)
    g96 = q8(np.concatenate([G / HD, Z24, G, Z24, G, Z24], axis=1))

    d_idx = np.arange(E) % HD
    bkd = kn_b[d_idx].astype(f32)
    qG = np.ascontiguousarray(
        np.concatenate([G / HD, G, G, G * bkd[:, None]], axis=1)).astype(f32)

    expand = np.ascontiguousarray(G.T).astype(f32)            # [8, 128]
    ident96b = np.eye(96, dtype=bf)
    ident128b = np.eye(128, dtype=bf)

    c1 = (SCALING * qn_w[d_idx] * kn_w[d_idx]).astype(f32).reshape(128, 1)
    c2 = (SCALING * qn_b[d_idx] * kn_w[d_idx]).astype(f32).reshape(128, 1)
    c3 = (SCALING * qn_w[d_idx]).astype(f32).reshape(128, 1)
    c4 = (SCALING * qn_b[d_idx]).astype(f32).reshape(128, 1)

    lam = float(np.exp(np.sum(lambda_q1 * lambda_k1))
                - np.exp(np.sum(lambda_q2 * lambda_k2)) + LAMBDA_INIT)
    lamrow = np.tile(np.array([1.0, lam] * 4, f32), (128, 1))

    global HAS_QC
    HAS_QC = bool(np.any(kn_b != 0.0))

    wdict = dict(w1dr=q8(w1dr), w2dr=q8(w2dr), wk8=wk8, g96=g96, qG=qG,
                 expand=expand, ident96b=ident96b, ident128b=ident128b,
                 c1=c1, c2=c2, c3=c3, c4=c4, lamrow=lamrow)
    WSPEC.clear()
    dtmap = {np.dtype(np.float32): F32, np.dtype(bf): BF16,
             np.dtype(e4): F8}
    for k, v in wdict.items():
        WSPEC[k] = (v.shape, dtmap[v.dtype])
    return wdict


def pack_x(x):
    """x [N, 56, 128] f32 -> (xe [N//2, 128, 160] f8, xt [N//2, 128, 2, 56])."""
    import ml_dtypes
    e4 = ml_dtypes.float8_e4m3
    xf = np.asarray(x, np.float32)
    x8 = np.clip(xf, -240, 240).astype(e4)
    n = x8.shape[0]
    xe = np.zeros((n // 16, 128, XW, 8), e4)
    ev = x8[0::2].reshape(n // 16, 8, A, E)    # [g, pair, a, e]
    od = x8[1::2].reshape(n // 16, 8, A, E)
    xe[:, 0:56, 3:3 + E, :] = ev.transpose(0, 2, 3, 1)
    xe[:, 64:120, 3:3 + E, :] = od.transpose(0, 2, 3, 1)
    bfd = ml_dtypes.bfloat16
    xb = xf.astype(bfd)
    xt = np.zeros((n // 2, 128, 2, A), bfd)
    xt[:, :, 0, :] = xb[0::2].transpose(0, 2, 1)
    xt[:, :, 1, :] = xb[1::2].transpose(0, 2, 1)
    return xe, xt


_CACHED = {}


def kernel(**inputs):
    xe, xt = pack_x(inputs["x"])
    wdict = prepare_weights(
        **{k: np.asarray(v, np.float32) for k, v in inputs.items()
           if k != "x"})
    if "nc" not in _CACHED:
        _CACHED["nc"] = build_nc()
    nc = _CACHED["nc"]
    nbp = NB // 2
    in_maps = []
    for c in range(NCORES):
        m = {"x": np.ascontiguousarray(xe[c * nbp:(c + 1) * nbp]),
             "xt": np.ascontiguousarray(xt[c * nbp:(c + 1) * nbp])}
        m.update(wdict)
        in_maps.append(m)
    res = run_bass_kernel_spmd(nc, in_maps, core_ids=list(range(NCORES)))
    return np.concatenate([np.asarray(r["out"]) for r in res.results], axis=0)


if __name__ == "__main__":
    import reference
    inputs = {k: np.asarray(v) for k, v in reference.setup_inputs().items()}
    got = kernel(**inputs)
    exp = np.asarray(reference.reference(**inputs))
    err = np.abs(got - exp).max() / np.abs(exp).max()
    print("rel err:", err)


# revision 30
# speedup vs baseline: 1.2490x; 1.0463x over previous
"""Trainium2 Bass kernel for nn_AtlasMultiDiffAttn (8-core data-parallel).

Self-contained: hardcodes shapes (x [8192,56,128] f32 -> out [8192,56] f32).
Per core: 1024 samples, 8 tiles of BT=128 samples (64 even/odd sample pairs).

v2: fp8 (e4m3, TRN +-240) DoubleRow convs.
  - conv1/conv2 matmuls run fp8 DoubleRow (2 taps per instruction, Ko step=1
    overlapping-window rhs - verified on HW), 0.5 cyc/row.
  - x arrives in two host layouts: pair-image xe [pair, row, 160] fp8 for
    conv1 (rows 0-55 even / 64-119 odd sample channels, col c = x[., c-3]),
    and xt [pair, e, 2, 56] fp8 for k-proj (no on-device x transposes).
  - scales: w1 x16, w2 x16, wk x8 keep fp8 weights out of subnormals;
    h = 32*silu (<240), k_T = 8*k; layernorms absorb scales via
    EPS_Q = (1024*56)^2*EPS and EPS_K = 64*EPS.
  - silu via tanh (exp_and_others act table; no table switches):
    2silu(y) = (tanh(y/2)+1)*y with y held scaled in PSUM.
  - h_T via PE transposes (fp8); stats/scores as in v1 but fp8/bf16 operands.
"""
from contextlib import ExitStack

import numpy as np

import concourse.bass as bass
import concourse.tile as tile
from concourse import bacc, mybir
from concourse.bass_utils import run_bass_kernel_spmd

F32 = mybir.dt.float32
BF16 = mybir.dt.bfloat16
F8 = mybir.dt.float8e4
I32 = mybir.dt.int32
AF = mybir.ActivationFunctionType
OP = mybir.AluOpType
AX = mybir.AxisListType
DR = mybir.MatmulPerfMode.DoubleRow

B, A, E = 8192, 56, 128
H, HD = 4, 16
LAMBDA_INIT = 0.7
EPS = 1e-5
SCALING = HD ** -0.5

W1S = 16.0                # conv1 weight quant scale
W2S = 16.0                # conv2 weight quant scale
WKS = 8.0                 # k-proj weight quant scale
QC_SCALE = float(A)              # q_acc = QC_SCALE * q_true
EPS_Q = QC_SCALE * QC_SCALE * EPS
EXP_A = 12102203.161561485       # 2^23 / ln 2 (Schraudolph fast exp)
EXP_B = 1065353216.0 - 361007.0  # 127*2^23 - C (rms-optimal bias)
EPS_K = WKS * WKS * EPS

NCORES = 8
NB = B // NCORES          # 1024 samples per core
BT = 128                  # samples per tile
NT = NB // BT             # 8 tiles
NPAIR = BT // 2           # 64
XW = 136                  # padded conv1 col count (cols 3..130 = x)
MAGIC = 0x5F3759DF

WSPEC = {}
HAS_QC = True


def _newton_rsqrt(nc, pool, v_ap, shape, tag):
    """v_ap <- rsqrt(v_ap): magic init + 1 Newton iteration (~0.2% rel)."""
    y = pool.tile(list(shape), F32, tag=f"nwy_{tag}")
    t = pool.tile(list(shape), F32, tag=f"nwt_{tag}")
    npart = v_ap.shape[0]
    ya, ta = y[0:npart], t[0:npart]
    nc.vector.tensor_scalar(out=ya.bitcast(I32), in0=v_ap.bitcast(I32),
                            scalar1=1, scalar2=None,
                            op0=OP.logical_shift_right)
    nc.vector.tensor_scalar(out=ya.bitcast(I32), in0=ya.bitcast(I32),
                            scalar1=-1, scalar2=MAGIC,
                            op0=OP.mult, op1=OP.add)
    nc.vector.tensor_tensor(out=ta, in0=ya, in1=ya, op=OP.mult)
    nc.vector.tensor_tensor(out=ta, in0=ta, in1=v_ap, op=OP.mult)
    nc.vector.tensor_scalar(out=ta, in0=ta, scalar1=-0.5, scalar2=1.5,
                            op0=OP.mult, op1=OP.add)
    nc.vector.tensor_tensor(out=v_ap, in0=ya, in1=ta, op=OP.mult)


def apx(base, insert, offset_add):
    """Append (stride,count) dims to an AP and bump its offset."""
    return bass.AP(tensor=base.tensor, offset=base.offset + offset_add,
                   ap=list(base.ap) + [list(d) for d in insert])


def build_tile_kernel(ctx, tc, xe_ext, xt_ext, out_ext, wext, reps=1):
    nc = tc.nc

    consts = ctx.enter_context(tc.tile_pool(name="consts", bufs=1))
    sbX = ctx.enter_context(tc.tile_pool(name="sbX", bufs=2))
    sbT = ctx.enter_context(tc.tile_pool(name="sbT", bufs=2))
    sbH = ctx.enter_context(tc.tile_pool(name="sbH", bufs=1))
    sbK = ctx.enter_context(tc.tile_pool(name="sbK", bufs=2))
    sbS = ctx.enter_context(tc.tile_pool(name="sbS", bufs=2))
    sb2 = ctx.enter_context(tc.tile_pool(name="sb2", bufs=2))
    sbQ = ctx.enter_context(tc.tile_pool(name="sbQ", bufs=1))
    psA = ctx.enter_context(tc.tile_pool(name="psA", bufs=2, space="PSUM"))
    psB = ctx.enter_context(tc.tile_pool(name="psB", bufs=2, space="PSUM"))
    psC = ctx.enter_context(tc.tile_pool(name="psC", bufs=2, space="PSUM"))

    def cload(name):
        shape, pdt = WSPEC[name]
        t = consts.tile(list(shape), pdt, tag=f"c_{name}")
        nc.sync.dma_start(out=t[:], in_=wext[name][:])
        return t

    w1 = cload("w1dr")          # [128, 4, 2, 128] f8
    w2 = cload("w2dr")          # [128, 4, 2, 128] f8
    wk = cload("wk8")           # [128, 128] f8
    g96 = cload("g96")          # [128, 96] f8
    qg = cload("qG")            # [128, 32] f32
    expd = cload("expand")      # [8, 128] f32
    idb = cload("ident96b")     # [96, 96] bf16
    id128 = cload("ident128b")  # [128, 128] bf16
    c1 = cload("c1")            # [128, 1] f32
    c2 = cload("c2")
    c3 = cload("c3")
    c4 = cload("c4")
    lamrow = cload("lamrow")    # [128, 8] f32

    def make_tile(it):
        p0g = it * NPAIR

        # pair-minor octet layout: x_sb[r, chunk, col, pair] so the DR rhs
        # flattens (col, pair) into one contiguous dim (3D AP requirement)
        x_sb = sbX.tile([128, 8, XW, 8], F8, tag="xsb")
        g0 = it * 8
        for qd in range(4):
            q0 = qd * 2
            nc.sync.dma_start(
                out=x_sb[:, q0:q0 + 2, :, :],
                in_=xe_ext[g0 + q0:g0 + q0 + 2].transpose([1, 0, 2, 3]))
        x_T = sbT.tile([128, NPAIR, 2, A], BF16, tag="xT")
        for hd in range(2):
            h0 = hd * (NPAIR // 2)
            nc.sync.dma_start(
                out=x_T[:, h0:h0 + NPAIR // 2, :, :],
                in_=xt_ext[p0g + h0:p0g + h0 + NPAIR // 2]
                    .transpose([1, 0, 2, 3]))

        h = sbH.tile([128, 8, E, 8], BF16, tag="h")
        h_T = sbH.tile([128, 8, E, 8], F8, tag="hT")
        k_T = sbK.tile([128, NPAIR, 112], BF16, tag="kT")
        k2 = sbK.tile([128, NPAIR, 112], BF16, tag="k2")
        qbk = sbQ.tile([128, NPAIR, 112], BF16, tag="qbk")
        q_acc = sb2.tile([128, 128], F32, tag="qacc")
        st = dict(it=it, k_T=k_T, k2=k2, qbk=qbk, q_acc=q_acc)

        xb = x_sb[0:120, 0:1, 0:1, 0:1].squeeze(3).squeeze(2).squeeze(1)
        hb = h_T[:, 0:1, 0:1, 0:1].squeeze(3).squeeze(2).squeeze(1)

        def conv1_chunk(c):
            ps = psA.tile([128, 1024], F32, tag="ps1")
            for ti in range(4):
                for hf in range(2):
                    rhs = apx(xb, [[8, 2], [1, 512]],
                              c * (XW * 8) + 16 * ti + 512 * hf)
                    nc.tensor.matmul(ps[:, 512 * hf:512 * (hf + 1)],
                                     w1[0:120, ti], rhs,
                                     start=(ti == 0), stop=(ti == 3),
                                     perf_mode=DR)
            nc.scalar.activation(
                h[:, c].rearrange("p e q -> p (e q)"), ps[:],
                AF.Silu, scale=1.0 / W1S)

        def transpose_block(c):
            for hf in range(2):
                psx = psC.tile([128, 4, E], BF16, tag="psb")
                for i in range(4):
                    pr = 4 * hf + i
                    nc.tensor.transpose(
                        psx[:, i, :],
                        h[:, c, :, pr:pr + 1].squeeze(2), id128[:])
                dst = h_T[:, c, :, 4 * hf:4 * hf + 4].transpose([0, 2, 1])
                nc.scalar.copy(out=dst, in_=psx[:])

        def conv2_chunk(c):
            ps2 = psA.tile([128, 1024], F32, tag="ps1")
            for ti in range(4):
                for eo in range(2):
                    rhs = apx(hb, [[8, 2], [1, 448]],
                              c * (E * 8) + (62 * eo + 2 * ti) * 8)
                    nc.tensor.matmul(ps2[:, 512 * eo:512 * eo + 448],
                                     w2[:, ti], rhs,
                                     start=(ti == 0), stop=(ti == 3),
                                     perf_mode=DR)
            ps2_b = ps2[:, 0:1].squeeze(1)
            ps2v = apx(ps2_b, [[512, 2], [1, 448]], 0)
            h2s = sb2.tile([128, 896], BF16, tag="h2s")
            nc.scalar.activation(h2s[:], ps2v, AF.Silu,
                                 scale=1.0 / W2S)
            # q accumulation: h2s cols are (eo, a, pair); q_acc cols are
            # sample-ordered (pair-major, eo minor) -> strided out view
            qa_b = q_acc[:, 0:1].squeeze(1)
            h2_b = h2s[:, 0:1].squeeze(1)
            nc.vector.reduce_sum(
                apx(qa_b, [[1, 2], [2, 8]], 16 * c),
                apx(h2_b, [[448, 2], [1, 8], [8, A]], 0),
                axis=AX.X)

        def kproj_block(c):
            for hf in range(2):
                b = 2 * c + hf
                psk = psB.tile([128, 512], F32, tag="psk")
                rhs = x_T[:, 4 * b:4 * b + 4, :, :].rearrange(
                    "p q c a -> p (q c a)")
                nc.tensor.matmul(psk[:, 0:448], wk[:], rhs,
                                 start=True, stop=True)
                dst = k_T[:, 4 * b:4 * b + 4, :].rearrange(
                    "p q r -> p (q r)")
                nc.scalar.copy(out=dst, in_=psk[:, 0:448])
            p0 = c * 8
            nc.gpsimd.tensor_tensor(
                out=k2[:, p0:p0 + 8, :], in0=k_T[:, p0:p0 + 8, :],
                in1=k_T[:, p0:p0 + 8, :], op=OP.mult)

        def step(s):
            if s < 8:
                conv1_chunk(s)
            if 1 <= s <= 8:
                transpose_block(s - 1)
            if s >= 2:
                conv2_chunk(s - 2)
            if s < 8:
                kproj_block(s)

        return step, st

    def make_tail(st):
        it = st["it"]
        k_T, k2, qbk, q_acc = st["k_T"], st["k2"], st["qbk"], st["q_acc"]
        sh = {}

        def p0():
            qpsA = psB.tile([128, 512], F32, tag="psk")
            nc.tensor.matmul(qpsA[0:8, 0:128], qg[:, 0:8], q_acc[:],
                             start=True, stop=True)           # muq
            q2 = sbQ.tile([128, 128], F32, tag="q2")
            nc.scalar.activation(q2[:], q_acc[:], AF.Square)
            nc.tensor.matmul(qpsA[0:8, 128:256], qg[:, 8:16], q2[:],
                             start=True, stop=True)           # sum q^2
            mq2 = sbQ.tile([128, 256], F32, tag="mq2")
            nc.vector.tensor_copy(out=mq2[0:8, :], in_=qpsA[0:8, 0:256])
            sh["mq2"] = mq2

        def p1():
            mq2 = sh["mq2"]
            vq = sbQ.tile([128, 128], F32, tag="vq")
            nc.vector.tensor_tensor(out=vq[0:8, :], in0=mq2[0:8, 0:128],
                                    in1=mq2[0:8, 0:128], op=OP.mult)
            nc.vector.scalar_tensor_tensor(
                out=vq[0:8, :], in0=mq2[0:8, 128:256], scalar=1.0 / HD,
                in1=vq[0:8, :], op0=OP.mult, op1=OP.subtract)
            nc.vector.tensor_scalar_add(vq[0:8, :], vq[0:8, :], EPS_Q)
            _newton_rsqrt(nc, sbQ, vq[0:8, :], [128, 128], "rq")
            sh["vq"] = vq

        def p2():
            mq2, vq = sh["mq2"], sh["vq"]
            qpsA = psB.tile([128, 512], F32, tag="psk")
            nc.tensor.matmul(qpsA[:, 0:128], expd[:], mq2[0:8, 0:128],
                             start=True, stop=True)
            nc.tensor.matmul(qpsA[:, 128:256], expd[:], vq[0:8, :],
                             start=True, stop=True)
            qhat = sbQ.tile([128, 128], F32, tag="qhat")
            nc.vector.tensor_tensor(out=qhat[:], in0=q_acc[:],
                                    in1=qpsA[:, 0:128], op=OP.subtract)
            nc.vector.tensor_tensor(out=qhat[:], in0=qhat[:],
                                    in1=qpsA[:, 128:256], op=OP.mult)
            qb = sbQ.tile([128, 128], F32, tag="qb")
            nc.vector.tensor_scalar(out=qb[:], in0=qhat[:], scalar1=c1[:],
                                    scalar2=c2[:], op0=OP.mult, op1=OP.add)
            sh["qhat"], sh["qb"] = qhat, qb

        def p3():
            qhat, qb = sh["qhat"], sh["qb"]
            qpsB = psB.tile([128, 512], F32, tag="psk")
            nc.tensor.matmul(qpsB[0:8, 0:128], qg[:, 16:24], qb[:],
                             start=True, stop=True)           # QB
            if HAS_QC:
                qa = sbQ.tile([128, 128], F32, tag="qa")
                nc.vector.tensor_scalar(out=qa[:], in0=qhat[:],
                                        scalar1=c3[:], scalar2=c4[:],
                                        op0=OP.mult, op1=OP.add)
                nc.tensor.matmul(qpsB[32:40, 128:256], qg[:, 24:32], qa[:],
                                 start=True, stop=True,
                                 tile_position=(0, 32))       # QC
            qcat = sbQ.tile([64, 128], BF16, tag="qcat")
            nc.vector.memset(qcat[:], 0.0)
            nc.vector.tensor_copy(out=qcat[0:8, :], in_=qpsB[0:8, 0:128])
            if HAS_QC:
                nc.vector.tensor_copy(out=qcat[32:40, :],
                                      in_=qpsB[32:40, 128:256])
            qps = psC.tile([128, 64], BF16, tag="psb")
            nc.tensor.transpose(qps[:, 0:64], qcat[0:64, :],
                                idb[0:64, 0:64])
            qsb = sb2.tile([128, 64], F32, tag="qsb")
            nc.vector.tensor_copy(out=qsb[:], in_=qps[:, 0:64])
            qbb = sbQ.tile([128, 128], BF16, tag="qbb")
            nc.gpsimd.tensor_copy(out=qbb[:], in_=qb[:])
            sh["qsb"], sh["qbb"] = qsb, qbb

        def qbk_piece(lo, hi):
            def f():
                qbb = sh["qbb"]
                for cki in range(lo, hi):
                    p0_ = cki * 8
                    nc.gpsimd.tensor_tensor(
                        out=qbk[:, p0_:p0_ + 8, :],
                        in0=k_T[:, p0_:p0_ + 8, :].rearrange(
                            "p q (c a) -> p q c a", c=2),
                        in1=qbb[:, 2 * p0_:2 * p0_ + 16]
                            .rearrange("p (q c) -> p q c", c=2).unsqueeze(3)
                            .to_broadcast((128, 8, 2, A)),
                        op=OP.mult)
            return f

        def stats_alloc():
            stats_sb = sb2.tile([128, NPAIR, 112], BF16, tag="statsb")
            sh["stats_sb"] = stats_sb

        def stats_piece(lo, hi):
            def f():
                stats_sb = sh["stats_sb"]
                for cki in range(lo, hi):
                    p0_ = cki * 4
                    pst = psB.tile([128, 512], F32, tag="psk")
                    nc.tensor.matmul(pst[0:32, 0:448], g96[:, 0:32],
                                     k_T[:, p0_:p0_ + 4, :],
                                     start=True, stop=True)
                    nc.tensor.matmul(pst[32:64, 0:448], g96[:, 32:64],
                                     k2[:, p0_:p0_ + 4, :],
                                     start=True, stop=True,
                                     tile_position=(0, 32))
                    nc.tensor.matmul(pst[64:96, 0:448], g96[:, 64:96],
                                     qbk[:, p0_:p0_ + 4, :],
                                     start=True, stop=True,
                                     tile_position=(0, 64))
                    dst = stats_sb[0:96, p0_:p0_ + 4, :].rearrange(
                        "p q r -> p (q r)")
                    if cki % 2 == 0:
                        nc.vector.tensor_copy(out=dst, in_=pst[0:96, 0:448])
                    else:
                        nc.scalar.copy(out=dst, in_=pst[0:96, 0:448])
            return f

        def statsB_piece(lo, hi):
            def f():
                if "statsB" not in sh:
                    statsB_t = sbQ.tile([128, A, 96], BF16, tag="statsB")
                    sh["statsB"] = statsB_t
                statsB = sh["statsB"]
                svb = sh["stats_sb"][:].rearrange("p q (c l) -> p (q c) l",
                                                  c=2)
                for li in range(lo, hi):
                    l0 = li * 7
                    pstb = psC.tile([128, 7, 96], BF16, tag="psb")
                    for j in range(7):
                        nc.tensor.transpose(pstb[:, j, :],
                                            svb[0:96, :, l0 + j],
                                            idb[0:96, 0:96])
                    if li % 2 == 0:
                        nc.vector.tensor_copy(out=statsB[:, l0:l0 + 7, :],
                                              in_=pstb[:])
                    else:
                        nc.scalar.copy(out=statsB[:, l0:l0 + 7, :],
                                       in_=pstb[:])
            return f

        def p_asm():
            statsB = sh["statsB"]
            qsb = sh["qsb"]
            muk = statsB[:, :, 0:8]
            sk2 = statsB[:, :, 32:40]
            QK = statsB[:, :, 64:72]
            vk = sbQ.tile([128, A, 8], F32, tag="vk")
            nc.vector.tensor_tensor(out=vk[:], in0=muk, in1=muk, op=OP.mult)
            nc.vector.scalar_tensor_tensor(out=vk[:], in0=sk2,
                                           scalar=1.0 / HD, in1=vk[:],
                                           op0=OP.mult, op1=OP.subtract)
            nc.vector.tensor_scalar_add(vk[:], vk[:], EPS_K)
            _newton_rsqrt(nc, sbQ, vk[:], [128, A, 8], "rk")
            s_sc = sbQ.tile([128, A, 8], F32, tag="ssc")
            QBb = qsb[:, 0:8].unsqueeze(1).to_broadcast((128, A, 8))
            QCb = qsb[:, 32:40].unsqueeze(1).to_broadcast((128, A, 8))
            nc.vector.tensor_tensor(out=s_sc[:], in0=muk, in1=QBb,
                                    op=OP.mult)
            nc.vector.tensor_tensor(out=s_sc[:], in0=QK, in1=s_sc[:],
                                    op=OP.subtract)
            nc.vector.tensor_tensor(out=s_sc[:], in0=s_sc[:], in1=vk[:],
                                    op=OP.mult)
            if HAS_QC:
                nc.vector.tensor_tensor(out=s_sc[:], in0=s_sc[:], in1=QCb,
                                        op=OP.add)
            sh["s_sc"] = s_sc

        def p_sm():
            s_sc = sh["s_sc"]
            # scores are bounded (|s| = O(4)): exp cannot overflow, so
            # skip the max-subtraction pass (softmax is shift-invariant).
            # exp via tanh (the silu act table has no exp):
            # e^s = (1+tanh(s/2)) / (1-tanh(s/2))
            se = sbQ.tile([128, A, 8], F32, tag="se")
            nc.scalar.activation(se[:], s_sc[:], AF.Tanh, scale=0.5)
            sb_ = sbQ.tile([128, A, 8], F32, tag="sb_")
            nc.vector.tensor_scalar(out=sb_[:], in0=se[:], scalar1=-1.0,
                                    scalar2=1.0, op0=OP.mult, op1=OP.add)
            nc.vector.reciprocal(sb_[:], sb_[:])
            nc.vector.tensor_scalar_add(se[:], se[:], 1.0)
            nc.vector.tensor_tensor(out=s_sc[:], in0=se[:], in1=sb_[:],
                                    op=OP.mult)
            z1 = sbQ.tile([128, 8], F32, tag="z1")
            nc.vector.reduce_sum(z1[:], s_sc[:].transpose([0, 2, 1]),
                                 axis=AX.X)
            rz1 = sbQ.tile([128, 8], F32, tag="rz1")
            nc.vector.reciprocal(rz1[:], z1[:])
            nc.vector.tensor_tensor(out=rz1[:], in0=rz1[:], in1=lamrow[:],
                                    op=OP.mult)
            nc.vector.tensor_tensor(
                out=s_sc[:], in0=s_sc[:],
                in1=rz1[:].unsqueeze(1).to_broadcast((128, A, 8)),
                op=OP.mult)
            dd = sbQ.tile([128, A, 4], F32, tag="dd")
            nc.vector.tensor_tensor(out=dd[:], in0=s_sc[:, :, 0:8:2],
                                    in1=s_sc[:, :, 1:8:2], op=OP.subtract)
            de = sbQ.tile([128, A, 4], F32, tag="de")
            nc.scalar.activation(de[:], dd[:], AF.Tanh, scale=0.5)
            db_ = sbQ.tile([128, A, 4], F32, tag="db_")
            nc.vector.tensor_scalar(out=db_[:], in0=de[:], scalar1=-1.0,
                                    scalar2=1.0, op0=OP.mult, op1=OP.add)
            nc.vector.reciprocal(db_[:], db_[:])
            nc.vector.tensor_scalar_add(de[:], de[:], 1.0)
            nc.vector.tensor_tensor(out=dd[:], in0=de[:], in1=db_[:],
                                    op=OP.mult)
            z2 = sbQ.tile([128, 4], F32, tag="z2")
            nc.vector.reduce_sum(z2[:], dd[:].transpose([0, 2, 1]),
                                 axis=AX.X)
            rz2 = sbQ.tile([128, 4], F32, tag="rz2")
            nc.vector.reciprocal(rz2[:], z2[:])
            nc.vector.tensor_scalar_mul(rz2[:], rz2[:], 1.0 / H)
            nc.vector.tensor_tensor(
                out=dd[:], in0=dd[:],
                in1=rz2[:].unsqueeze(1).to_broadcast((128, A, 4)),
                op=OP.mult)
            ot = sbQ.tile([128, A], F32, tag="ot")
            nc.vector.reduce_sum(ot[:], dd[:], axis=AX.X)
            nc.sync.dma_start(out=out_ext[it * BT:(it + 1) * BT, :],
                              in_=ot[:])

        def p_stats_alloc_and_first():
            stats_alloc()
            stats_piece(0, 4)()

        early = [p0, p1, p2, p3,
                 qbk_piece(0, 4), qbk_piece(4, 8),
                 p_stats_alloc_and_first,
                 stats_piece(4, 9), stats_piece(9, 13), stats_piece(13, 16)]
        late = [statsB_piece(0, 4), statsB_piece(4, 8), p_asm, p_sm]
        return early, late

    E_SLOTS = [0, 1, 2, 3, 4, 5, 6, 7, 8, 9]
    L_SLOTS = [0, 1, 2, 4]
    pend_E, pend_L = [], []
    for it_ in range(NT * reps):
        step, st = make_tile(it_ % NT)
        sched = {}
        for p, s in zip(pend_L, L_SLOTS):
            sched.setdefault(s, []).append(p)
        for p, s in zip(pend_E, E_SLOTS):
            sched.setdefault(s, []).append(p)
        for s in range(10):
            step(s)
            for piece in sched.get(s, []):
                piece()
        new_E, new_L = make_tail(st)
        pend_L = pend_E_L if (pend_E_L := getattr(make_tile, "_carryL", None)) else []
        make_tile._carryL = new_L
        pend_E = new_E
    for piece in pend_L:
        piece()
    for piece in pend_E:
        piece()
    for piece in getattr(make_tile, "_carryL", []):
        piece()


def build_nc(reps=1):
    nc = bacc.Bacc(target_bir_lowering=False, trn_type="TRN2")
    xe_ext = nc.declare_dram_parameter("x", [NB // 16, 128, XW, 8], F8,
                                       isOutput=False)
    xt_ext = nc.declare_dram_parameter("xt", [NB // 2, 128, 2, A], BF16,
                                       isOutput=False)
    out_ext = nc.declare_dram_parameter("out", [NB, A], F32, isOutput=True)
    wext = {}
    for name, (shape, dt) in WSPEC.items():
        wext[name] = nc.declare_dram_parameter(name, list(shape), dt,
                                               isOutput=False)
    with tile.TileContext(nc) as tc:
        with ExitStack() as ctx:
            build_tile_kernel(ctx, tc, xe_ext, xt_ext, out_ext, wext,
                              reps=reps)
    nc.compile()
    return nc


def prepare_weights(w_emb, b_emb, w_atlas, b_atlas, w_k, qn_w, qn_b, kn_w,
                    kn_b, lambda_q1, lambda_k1, lambda_q2, lambda_k2):
    import ml_dtypes
    bf = ml_dtypes.bfloat16
    e4 = ml_dtypes.float8_e4m3
    f32 = np.float32

    assert np.allclose(b_atlas, 0.0), "kernel assumes b_atlas == 0"
    assert np.allclose(b_emb, 0.0), "kernel assumes b_emb == 0"

    def q8(v):
        return np.clip(v, -240.0, 240.0).astype(e4)

    # conv1 lhsT [120, (ti,j)=tap, 128]: contract rows match pair-image rows
    # (0-55 even, 64-119 odd); out cols match h col windows (3:59 / 65:121)
    w1dr = np.zeros((128, 4, 2, 128), f32)
    for t in range(7):
        blk = W1S * np.transpose(w_emb[:, :, t]).astype(f32)
        w1dr[0:56, t // 2, t % 2, 3:59] = blk
        w1dr[64:120, t // 2, t % 2, 65:121] = blk

    # conv2 lhsT [e_in, tap, e_out]
    w2dr = np.zeros((128, 4, 2, 128), f32)
    for t in range(7):
        w2dr[:, t // 2, t % 2, :] = W2S * np.transpose(
            w_atlas[:, :, t]).astype(f32)

    wk8 = q8(WKS * np.transpose(w_k))

    G = np.zeros((128, 8), f32)
    for o in range(128):
        G[o, o // HD] = 1.0
    Z24 = np.zeros((128, 24), f32)
    g96 = q8(np.concatenate([G / HD, Z24, G, Z24, G, Z24], axis=1))

    d_idx = np.arange(E) % HD
    bkd = kn_b[d_idx].astype(f32)
    qG = np.ascontiguousarray(
        np.concatenate([G / HD, G, G, G * bkd[:, None]], axis=1)).astype(f32)

    expand = np.ascontiguousarray(G.T).astype(f32)            # [8, 128]
    ident96b = np.eye(96, dtype=bf)
    ident128b = np.eye(128, dtype=bf)

    c1 = (SCALING * qn_w[d_idx] * kn_w[d_idx]).astype(f32).reshape(128, 1)
    c2 = (SCALING * qn_b[d_idx] * kn_w[d_idx]).astype(f32).reshape(128, 1)
    c3 = (SCALING * qn_w[d_idx]).astype(f32).reshape(128, 1)
    c4 = (SCALING * qn_b[d_idx]).astype(f32).reshape(128, 1)

    lam = float(np.exp(np.sum(lambda_q1 * lambda_k1))
                - np.exp(np.sum(lambda_q2 * lambda_k2)) + LAMBDA_INIT)
    lamrow = np.tile(np.array([1.0, lam] * 4, f32), (128, 1))

    global HAS_QC
    HAS_QC = bool(np.any(kn_b != 0.0))

    wdict = dict(w1dr=q8(w1dr), w2dr=q8(w2dr), wk8=wk8, g96=g96, qG=qG,
                 expand=expand, ident96b=ident96b, ident128b=ident128b,
                 c1=c1, c2=c2, c3=c3, c4=c4, lamrow=lamrow)
    WSPEC.clear()
    dtmap = {np.dtype(np.float32): F32, np.dtype(bf): BF16,
             np.dtype(e4): F8}
    for k, v in wdict.items():
        WSPEC[k] = (v.shape, dtmap[v.dtype])
    return wdict


def pack_x(x):
    """x [N, 56, 128] f32 -> (xe [N//2, 128, 160] f8, xt [N//2, 128, 2, 56])."""
    import ml_dtypes
    e4 = ml_dtypes.float8_e4m3
    xf = np.asarray(x, np.float32)
    x8 = np.clip(xf, -240, 240).astype(e4)
    n = x8.shape[0]
    xe = np.zeros((n // 16, 128, XW, 8), e4)
    ev = x8[0::2].reshape(n // 16, 8, A, E)    # [g, pair, a, e]
    od = x8[1::2].reshape(n // 16, 8, A, E)
    xe[:, 0:56, 3:3 + E, :] = ev.transpose(0, 2, 3, 1)
    xe[:, 64:120, 3:3 + E, :] = od.transpose(0, 2, 3, 1)
    bfd = ml_dtypes.bfloat16
    xb = xf.astype(bfd)
    xt = np.zeros((n // 2, 128, 2, A), bfd)
    xt[:, :, 0, :] = xb[0::2].transpose(0, 2, 1)
    xt[:, :, 1, :] = xb[1::2].transpose(0, 2, 1)
    return xe, xt


_CACHED = {}


def kernel(**inputs):
    xe, xt = pack_x(inputs["x"])
    wdict = prepare_weights(
        **{k: np.asarray(v, np.float32) for k, v in inputs.items()
           if k != "x"})
    if "nc" not in _CACHED:
        _CACHED["nc"] = build_nc()
    nc = _CACHED["nc"]
    nbp = NB // 2
    in_maps = []
    for c in range(NCORES):
        m = {"x": np.ascontiguousarray(xe[c * nbp:(c + 1) * nbp]),
             "xt": np.ascontiguousarray(xt[c * nbp:(c + 1) * nbp])}
        m.update(wdict)
        in_maps.append(m)
    res = run_bass_kernel_spmd(nc, in_maps, core_ids=list(range(NCORES)))
    return np.concatenate([np.asarray(r["out"]) for r in res.results], axis=0)


if __name__ == "__main__":
    import reference
    inputs = {k: np.asarray(v) for k, v in reference.setup_inputs().items()}
    got = kernel(**inputs)
    exp = np.asarray(reference.reference(**inputs))
    err = np.abs(got - exp).max() / np.abs(exp).max()
    print("rel err:", err)
